# revision 8
# baseline (speedup 1.0000x reference)
"""AdaClusteringAttention Trainium2 kernel (8 NeuronCores, data-parallel).

Shard batch B=32 across 8 cores (4 rows each); batch row 4i+j uses cluster
row j, so clusters are replicated to every core.

Per batch row (N=4096 tokens, D=64, C=129 clusters):
  1. One-hot segment sums via TensorE: lhsT=[Q|K] and [V|1] bf16 tiles
     (host-packed so the cast-DMA is contiguous per partition),
     rhs = one-hot O_t [128n x 129c] built on VectorE/GpSimd
     (iota + is_equal). PSUM accumulates seg^T [d, c] + exact f32 counts.
  2. Tiny [129]-sized attention math: w=1/counts, centers, QK matmuls,
     +ln(counts) folded into the QK PSUM via K=1 matmul (the weighted
     softmax reweighting for free), exp on ScalarE, row sums, A^T via PE
     transposes, AV matmuls.
  3. Gather: V'[c] = V_fin[c] - V_fin[128] makes cluster 128 implicit
     (K=128 exactly); lhsT=V' stationary, rhs=O^T [c, n] chunks; g is
     partition-broadcast by a K=1 matmul, copied PSUM->SBUF on ScalarE,
     one-hotted on VectorE (4x bf16 mode); gather output lands transposed
     [64, n] in PSUM; the V_fin[128] bias is added during the PSUM->SBUF
     copy (ScalarE Identity+bias); DMA out transposed, host un-transposes.
"""
import os
import sys

sys.path.insert(0, "/opt/trn_rl_repo")

import numpy as np

from concourse import bass, bacc, mybir
from concourse.bass_utils import run_bass_kernel_spmd
from concourse.tile import TileContext

F32 = mybir.dt.float32
BF16 = mybir.dt.bfloat16
I32 = mybir.dt.int32
AOT = mybir.AluOpType
ACTF = mybir.ActivationFunctionType

P = 128
C = 129          # clusters
N = 4096         # tokens per batch row
D = 64
NT = N // P      # 32 contraction tiles per row
ROWS = 4         # batch rows per core
NCORES = 8
CH = 512         # gather chunk width
NCH = N // CH    # 8 chunks


def build():
    nc = bacc.Bacc("TRN2", target_bir_lowering=False, debug=False,
                   num_devices=NCORES)
    qk_d = nc.declare_dram_parameter("qk", [ROWS, N, 2 * D], F32, isOutput=False)
    v1_d = nc.declare_dram_parameter("v1", [ROWS, N, D + 1], F32, isOutput=False)
    g_d = nc.declare_dram_parameter("g", [ROWS, N], F32, isOutput=False)
    gb_d = nc.declare_dram_parameter("gb", [ROWS, N], BF16, isOutput=False)
    outT_d = nc.declare_dram_parameter("outT", [ROWS // 2, P, N], F32, isOutput=True)
    acol_d = nc.declare_dram_parameter("acol", [C, ROWS], F32, isOutput=True)

    with TileContext(nc) as tc:
        with (
            tc.tile_pool(name="const", bufs=1) as cp,
            tc.tile_pool(name="xin", bufs=2) as xp,
            tc.tile_pool(name="gin", bufs=2) as gp,
            tc.tile_pool(name="opool", bufs=8) as op_,
            tc.tile_pool(name="otpool", bufs=3) as otp,
            tc.tile_pool(name="small", bufs=2) as sp,
            tc.tile_pool(name="outp", bufs=2) as outp,
            tc.tile_pool(name="segps", bufs=1, space="PSUM") as segps,
            tc.tile_pool(name="smps", bufs=3, space="PSUM") as smps,
            tc.tile_pool(name="ggps", bufs=3, space="PSUM") as ggps,
        ):
            # ---- constants ----
            iota_row_i = cp.tile([P, C], I32)
            nc.gpsimd.iota(iota_row_i[:], pattern=[[1, C]], base=0,
                           channel_multiplier=0)
            iota_row = cp.tile([P, C], BF16)
            nc.vector.tensor_copy(out=iota_row[:], in_=iota_row_i[:])
            iota_col_i = cp.tile([P, 1], I32)
            nc.gpsimd.iota(iota_col_i[:], pattern=[[0, 1]], base=0,
                           channel_multiplier=1)
            iota_col = cp.tile([P, 1], F32)
            nc.vector.tensor_copy(out=iota_col[:], in_=iota_col_i[:])
            ident_bf = cp.tile([P, P], BF16)
            nc.gpsimd.memset(ident_bf[:], 0.0)
            nc.gpsimd.affine_select(out=ident_bf[:], in_=ident_bf[:],
                                    compare_op=AOT.not_equal, fill=1.0,
                                    base=0, pattern=[[-1, P]],
                                    channel_multiplier=1)
            ones_row_f = cp.tile([1, P], F32)
            nc.vector.memset(ones_row_f[:], 1.0)
            ones_row_bf = cp.tile([1, P], BF16)
            nc.vector.memset(ones_row_bf[:], 1.0)
            zero_col = cp.tile([P, 1], F32)
            nc.vector.memset(zero_col[:], 0.0)

            acol_main = cp.tile([P, ROWS], F32)
            acol_tail = cp.tile([1, ROWS], F32)

            def acopy(out, in_):
                # ScalarE copy as Identity+0-bias (avoid ACT table thrash)
                nc.scalar.activation(out=out, in_=in_, func=ACTF.Identity,
                                     bias=zero_col[0:out.shape[0], :],
                                     scale=1.0)

            for j in range(ROWS):
                par = j % 2          # parity: partition half of out_pair
                pb = 64 * par
                if par == 0:
                    out_pair = outp.tile([P, N], BF16, tag="out_pair")

                # ---- input DMAs (contiguous per partition) ----
                g_sb = gp.tile([P, NT], F32, tag="g_sb")
                nc.sync.dma_start(out=g_sb[:],
                                  in_=g_d[j].rearrange("(p t) -> p t", p=P))
                g_row = gp.tile([1, N], BF16, tag="g_row")
                nc.sync.dma_start(out=g_row[:], in_=gb_d[j][None, :])

                xqk = xp.tile([P, NT, 2 * D], BF16, tag="xqk")
                nc.gpsimd.dma_start(out=xqk[:],
                                    in_=qk_d[j].rearrange("(p t) d -> p t d", p=P))
                xv1 = xp.tile([P, NT, D + 1], BF16, tag="xv1")
                nc.gpsimd.dma_start(out=xv1[:],
                                    in_=v1_d[j].rearrange("(p t) d -> p t d", p=P))

                # ---- one-hot + segment-sum matmuls ----
                ps_qk = segps.tile([P, C], F32, tag="ps_qk")
                ps_v1 = segps.tile([D + 1, C], F32, tag="ps_v1")
                for t in range(NT):
                    o_t = op_.tile([P, C], BF16, tag="o_t")
                    eng = nc.vector if t % 2 == 0 else nc.gpsimd
                    eng.tensor_scalar(out=o_t[:], in0=iota_row[:],
                                      scalar1=g_sb[:, t:t + 1],
                                      scalar2=None, op0=AOT.is_equal)
                    nc.tensor.matmul(ps_qk[:], lhsT=xqk[:, t, :], rhs=o_t[:],
                                     start=(t == 0), stop=(t == NT - 1),
                                     skip_group_check=True)
                    nc.tensor.matmul(ps_v1[:], lhsT=xv1[:, t, :], rhs=o_t[:],
                                     start=(t == 0), stop=(t == NT - 1),
                                     skip_group_check=True)

                # ---- small attention math ----
                counts = sp.tile([1, C], F32, tag="counts")
                nc.vector.tensor_copy(out=counts[:], in_=ps_v1[D:D + 1, :])
                ceps = sp.tile([1, C], F32, tag="ceps")
                nc.vector.tensor_scalar(out=ceps[:], in0=counts[:],
                                        scalar1=1e-20, scalar2=None,
                                        op0=AOT.add)
                w_row = sp.tile([1, C], F32, tag="w_row")
                nc.vector.reciprocal(out=w_row[:], in_=ceps[:])
                lc_row = sp.tile([1, C], F32, tag="lc_row")
                nc.scalar.activation(out=lc_row[:], in_=ceps[:], func=ACTF.Ln)

                # broadcast w along partitions via K=1 f32 matmul
                ps_w = smps.tile([P, C], F32, tag="sm")
                nc.tensor.matmul(ps_w[:], lhsT=ones_row_f[:], rhs=w_row[:],
                                 start=True, stop=True, skip_group_check=True)
                w_bc = sp.tile([P, C], F32, tag="w_bc")
                acopy(w_bc[:], ps_w[:])

                # centers (transposed layout [d, c]), bf16
                qcT = sp.tile([D, C], BF16, tag="qcT")
                nc.vector.tensor_tensor(out=qcT[:], in0=ps_qk[0:D, :],
                                        in1=w_bc[0:D, :], op=AOT.mult)
                kcT = sp.tile([D, C], BF16, tag="kcT")
                nc.vector.tensor_tensor(out=kcT[:], in0=ps_qk[D:2 * D, :],
                                        in1=w_bc[D:2 * D, :], op=AOT.mult)
                vcT = sp.tile([D, C], BF16, tag="vcT")
                nc.vector.tensor_tensor(out=vcT[:], in0=ps_v1[0:D, :],
                                        in1=w_bc[0:D, :], op=AOT.mult)

                # S = Qc @ Kc^T + ln(counts)  -> [129 q, 129 k] in PSUM
                ps_S = smps.tile([P, C], F32, tag="sm")
                ps_S1 = smps.tile([1, C], F32, tag="sm")
                nc.tensor.matmul(ps_S[:], lhsT=qcT[:, 0:P], rhs=kcT[:],
                                 start=True, stop=False, skip_group_check=True)
                nc.tensor.matmul(ps_S[:], lhsT=ones_row_f[:], rhs=lc_row[:],
                                 start=False, stop=True, skip_group_check=True)
                nc.tensor.matmul(ps_S1[:], lhsT=qcT[:, P:C], rhs=kcT[:],
                                 start=True, stop=False, skip_group_check=True)
                nc.tensor.matmul(ps_S1[:], lhsT=ones_row_f[0:1, 0:1],
                                 rhs=lc_row[:], start=False, stop=True,
                                 skip_group_check=True)

                a_exp = sp.tile([P, C], BF16, tag="a_exp")
                nc.scalar.activation(out=a_exp[:], in_=ps_S[:], func=ACTF.Exp)
                a_expt = sp.tile([1, C], BF16, tag="a_expt")
                nc.scalar.activation(out=a_expt[:], in_=ps_S1[:], func=ACTF.Exp)

                # row sums and 1/sum
                rs = sp.tile([P, 1], F32, tag="rs")
                nc.vector.tensor_reduce(out=rs[:], in_=a_exp[:],
                                        axis=mybir.AxisListType.X, op=AOT.add)
                rinv = sp.tile([P, 1], F32, tag="rinv")
                nc.vector.reciprocal(out=rinv[:], in_=rs[:])
                rst = sp.tile([1, 1], F32, tag="rst")
                nc.vector.tensor_reduce(out=rst[:], in_=a_expt[:],
                                        axis=mybir.AxisListType.X, op=AOT.add)
                rinvt = sp.tile([1, 1], F32, tag="rinvt")
                nc.vector.reciprocal(out=rinvt[:], in_=rst[:])

                # second output: A_full[:, 0]
                nc.vector.tensor_scalar(out=acol_main[:, j:j + 1],
                                        in0=a_exp[:, 0:1], scalar1=rinv[:],
                                        scalar2=None, op0=AOT.mult)
                nc.vector.tensor_scalar(out=acol_tail[:, j:j + 1],
                                        in0=a_expt[:, 0:1], scalar1=rinvt[:],
                                        scalar2=None, op0=AOT.mult)

                # A^T via PE transposes (bf16 pass-through)
                ps_T1 = smps.tile([P, P], BF16, tag="sm")
                nc.tensor.transpose(ps_T1[:], in_=a_exp[:, 0:P], identity=ident_bf[:])
                ps_T2 = smps.tile([1, P], BF16, tag="sm")
                nc.tensor.transpose(ps_T2[:], in_=a_exp[:, P:C], identity=ident_bf[:])
                ps_T3 = smps.tile([P, 1], BF16, tag="sm")
                nc.tensor.transpose(ps_T3[:], in_=a_expt[:, 0:P],
                                    identity=ident_bf[0:1, 0:1])
                at_m = sp.tile([P, C], BF16, tag="at_m")
                acopy(at_m[:, 0:P], ps_T1[:])
                acopy(at_m[:, P:C], ps_T3[:])
                at_t = sp.tile([1, C], BF16, tag="at_t")
                acopy(at_t[:, 0:P], ps_T2[:])
                nc.vector.tensor_copy(out=at_t[:, P:C], in_=a_expt[:, P:C])

                # Vc with c on partitions via PE transposes
                ps_T5 = smps.tile([P, D], BF16, tag="sm")
                nc.tensor.transpose(ps_T5[:], in_=vcT[:, 0:P],
                                    identity=ident_bf[0:D, 0:D])
                ps_T6 = smps.tile([1, D], BF16, tag="sm")
                nc.tensor.transpose(ps_T6[:], in_=vcT[:, P:C],
                                    identity=ident_bf[0:D, 0:D])
                vc_m = sp.tile([P, D], BF16, tag="vc_m")
                acopy(vc_m[:], ps_T5[:])
                vc_t = sp.tile([1, D], BF16, tag="vc_t")
                acopy(vc_t[:], ps_T6[:])

                # V_sum = A_exp @ Vc  (unnormalized)
                ps_V = smps.tile([P, D], F32, tag="sm")
                ps_V1 = smps.tile([1, D], F32, tag="sm")
                nc.tensor.matmul(ps_V[:], lhsT=at_m[:, 0:P], rhs=vc_m[:],
                                 start=True, stop=False, skip_group_check=True)
                nc.tensor.matmul(ps_V[:], lhsT=at_t[:, 0:P], rhs=vc_t[:],
                                 start=False, stop=True, skip_group_check=True)
                nc.tensor.matmul(ps_V1[:], lhsT=at_m[:, P:C], rhs=vc_m[:],
                                 start=True, stop=False, skip_group_check=True)
                nc.tensor.matmul(ps_V1[:], lhsT=at_t[:, P:C], rhs=vc_t[:],
                                 start=False, stop=True, skip_group_check=True)

                # V_fin[128] (bias row); V' = rinv*Vsum - bias
                vfin = sp.tile([1, D], F32, tag="vfin")
                nc.vector.tensor_scalar(out=vfin[:], in0=ps_V1[:],
                                        scalar1=rinvt[:], scalar2=None,
                                        op0=AOT.mult)
                ps_vbc = smps.tile([P, D], F32, tag="sm")
                nc.tensor.matmul(ps_vbc[:], lhsT=ones_row_f[:], rhs=vfin[:],
                                 start=True, stop=True, skip_group_check=True)
                vbc = sp.tile([P, D], F32, tag="vbc")
                acopy(vbc[:], ps_vbc[:])
                vp = sp.tile([P, D], BF16, tag="vp")
                nc.vector.scalar_tensor_tensor(out=vp[:], in0=ps_V[:],
                                               scalar=rinv[:], in1=vbc[:],
                                               op0=AOT.mult, op1=AOT.subtract)
                # bias as a [64,1] column at this row's parity half
                ps_bias = smps.tile([P, 1], F32, tag="sm")
                nc.tensor.transpose(ps_bias[0:D, :], in_=vfin[:],
                                    identity=ones_row_f[0:1, 0:1])
                bias_c = sp.tile([P, 1], F32, tag="bias_c")
                nc.vector.tensor_copy(out=bias_c[pb:pb + D, :], in_=ps_bias[0:D, :])

                # ---- gather: out^T[d, n] = V'^T-gather + bias ----
                for e in range(NCH):
                    ps_gb = ggps.tile([P, CH], F32, tag="gg")
                    nc.tensor.matmul(ps_gb[:], lhsT=ones_row_bf[:],
                                     rhs=g_row[:, e * CH:(e + 1) * CH],
                                     start=True, stop=True,
                                     skip_group_check=True)
                    grep = otp.tile([P, CH], BF16, tag="grep")
                    acopy(grep[:], ps_gb[:])
                    ot = otp.tile([P, CH], BF16, tag="ot")
                    nc.vector.tensor_scalar(out=ot[:], in0=grep[:],
                                            scalar1=iota_col[:], scalar2=None,
                                            op0=AOT.is_equal)
                    ps_g = ggps.tile([P, CH], F32, tag="gg")
                    nc.tensor.matmul(ps_g[pb:pb + D, :], lhsT=vp[:], rhs=ot[:],
                                     start=True, stop=True,
                                     tile_position=(0, pb),
                                     skip_group_check=True)
                    nc.scalar.activation(out=out_pair[pb:pb + D,
                                                      e * CH:(e + 1) * CH],
                                         in_=ps_g[pb:pb + D, :],
                                         func=ACTF.Identity,
                                         bias=bias_c[pb:pb + D, :], scale=1.0)

                if par == 1:
                    nc.gpsimd.dma_start(out=outT_d[j // 2], in_=out_pair[:])

            nc.sync.dma_start(out=acol_d[0:P, :], in_=acol_main[:])
            nc.sync.dma_start(out=acol_d[P:C, :], in_=acol_tail[:])

    nc.compile()
    return nc


_nc_cache = None


def _get_nc():
    global _nc_cache
    if _nc_cache is None:
        _nc_cache = build()
    return _nc_cache


def _run(inputs, trace=False):
    import ml_dtypes
    q = np.asarray(inputs["queries"], dtype=np.float32)
    k = np.asarray(inputs["keys"], dtype=np.float32)
    v = np.asarray(inputs["values"], dtype=np.float32)
    cl = np.asarray(inputs["clusters"])
    g32 = cl.astype(np.float32)
    gb = g32.astype(ml_dtypes.bfloat16)  # ids <= 128, exact in bf16

    B = NCORES * ROWS
    qk = np.concatenate([q, k], axis=-1)                     # [B, N, 128]
    v1 = np.concatenate(
        [v, np.ones((B, N, 1), np.float32)], axis=-1)        # [B, N, 65]

    nc = _get_nc()
    in_maps = []
    for i in range(NCORES):
        in_maps.append({
            "qk": np.ascontiguousarray(qk[ROWS * i:ROWS * (i + 1)]),
            "v1": np.ascontiguousarray(v1[ROWS * i:ROWS * (i + 1)]),
            "g": g32,
            "gb": gb,
        })
    r = run_bass_kernel_spmd(nc, in_maps, list(range(NCORES)), trace=trace)
    res = r.results

    out = np.empty((B, N, D), np.float32)
    acol = np.empty((B, C), np.float32)
    for i in range(NCORES):
        oT = np.asarray(res[i]["outT"], np.float32)          # [2, 128, 4096]
        out[ROWS * i:ROWS * (i + 1)] = (
            oT.reshape(2, 2, D, N).transpose(0, 1, 3, 2).reshape(ROWS, N, D))
        acol[ROWS * i:ROWS * (i + 1)] = np.asarray(res[i]["acol"], np.float32).T
    return (out, acol), r


def kernel(**inputs):
    (out, acol), _ = _run(inputs, trace=False)
    return out, acol


# revision 13
# speedup vs baseline: 1.6617x; 1.6617x over previous
"""AdaClusteringAttention Trainium2 kernel (8 NeuronCores, data-parallel).

Shard batch B=32 across 8 cores (4 rows each); batch row 4i+j uses cluster
row j, so clusters are replicated to every core.

Per batch row (N=4096 tokens, D=64, C=129 clusters):
  1. One-hot segment sums via TensorE: lhsT=[Q|K] and [V|1] bf16 tiles
     (host-packed so the cast-DMA is contiguous per partition),
     rhs = one-hot O_t [128n x 129c] built on VectorE/GpSimd
     (iota + is_equal). PSUM accumulates seg^T [d, c] + exact f32 counts.
  2. Tiny [129]-sized attention math: w=1/counts, centers, QK matmuls,
     +ln(counts) folded into the QK PSUM via K=1 matmul (the weighted
     softmax reweighting for free), exp on ScalarE, row sums, A^T via PE
     transposes, AV matmuls.
  3. Gather: V'[c] = V_fin[c] - V_fin[128] makes cluster 128 implicit
     (K=128 exactly); lhsT=V' stationary, rhs=O^T [c, n] chunks; g is
     partition-broadcast by a K=1 matmul, copied PSUM->SBUF on ScalarE,
     one-hotted on VectorE (4x bf16 mode); gather output lands transposed
     [64, n] in PSUM; the V_fin[128] bias is added during the PSUM->SBUF
     copy (ScalarE Identity+bias); DMA out transposed, host un-transposes.
"""
import os
import sys

sys.path.insert(0, "/opt/trn_rl_repo")

import numpy as np

from concourse import bass, bacc, mybir
from concourse.bass_utils import run_bass_kernel_spmd
from concourse.tile import TileContext

F32 = mybir.dt.float32
BF16 = mybir.dt.bfloat16
I32 = mybir.dt.int32
AOT = mybir.AluOpType
ACTF = mybir.ActivationFunctionType

P = 128
C = 129          # clusters
N = 4096         # tokens per batch row
D = 64
NT = N // P      # 32 contraction tiles per row
ROWS = 4         # batch rows per core
NCORES = 8
CH = 512         # gather chunk width
NCH = N // CH    # 8 chunks


def build():
    nc = bacc.Bacc("TRN2", target_bir_lowering=False, debug=False,
                   num_devices=NCORES)
    qk_d = nc.declare_dram_parameter("qk", [ROWS, N, 2 * D], F32, isOutput=False)
    v1_d = nc.declare_dram_parameter("v1", [ROWS, N, D + 1], F32, isOutput=False)
    g_d = nc.declare_dram_parameter("g", [ROWS, N], F32, isOutput=False)
    gb_d = nc.declare_dram_parameter("gb", [ROWS, N], BF16, isOutput=False)
    outT_d = nc.declare_dram_parameter("outT", [ROWS // 2, P, N], F32, isOutput=True)
    acol_d = nc.declare_dram_parameter("acol", [C, ROWS], F32, isOutput=True)

    with TileContext(nc) as tc:
        with (
            tc.tile_pool(name="const", bufs=1) as cp,
            tc.tile_pool(name="xin", bufs=2) as xp,
            tc.tile_pool(name="gin", bufs=2) as gp,
            tc.tile_pool(name="opool", bufs=8) as op_,
            tc.tile_pool(name="otpool", bufs=3) as otp,
            tc.tile_pool(name="small", bufs=2) as sp,
            tc.tile_pool(name="outp", bufs=2) as outp,
            tc.tile_pool(name="segps", bufs=1, space="PSUM") as segps,
            tc.tile_pool(name="smps", bufs=3, space="PSUM") as smps,
            tc.tile_pool(name="ggps", bufs=3, space="PSUM") as ggps,
        ):
            # ---- constants ----
            # one-hot compare row padded to an even 130 cols (4x DVE mode)
            iota_row_i = cp.tile([P, C + 1], I32)
            nc.gpsimd.iota(iota_row_i[:], pattern=[[1, C + 1]], base=0,
                           channel_multiplier=0)
            iota_row = cp.tile([P, C + 1], BF16)
            nc.vector.tensor_copy(out=iota_row[:], in_=iota_row_i[:])
            iota_col_i = cp.tile([P, 1], I32)
            nc.gpsimd.iota(iota_col_i[:], pattern=[[0, 1]], base=0,
                           channel_multiplier=1)
            iota_col = cp.tile([P, 1], F32)
            nc.vector.tensor_copy(out=iota_col[:], in_=iota_col_i[:])
            ident_bf = cp.tile([P, P], BF16)
            nc.gpsimd.memset(ident_bf[:], 0.0)
            nc.gpsimd.affine_select(out=ident_bf[:], in_=ident_bf[:],
                                    compare_op=AOT.not_equal, fill=1.0,
                                    base=0, pattern=[[-1, P]],
                                    channel_multiplier=1)
            ones_row_f = cp.tile([1, P], F32)
            nc.vector.memset(ones_row_f[:], 1.0)
            ones_row_bf = cp.tile([1, P], BF16)
            nc.vector.memset(ones_row_bf[:], 1.0)
            zero_col = cp.tile([P, 1], F32)
            nc.vector.memset(zero_col[:], 0.0)

            acol_main = cp.tile([P, ROWS], F32)
            acol_tail = cp.tile([1, ROWS], F32)

            def acopy(out, in_):
                # ScalarE copy as Identity+0-bias (avoid ACT table thrash)
                nc.scalar.activation(out=out, in_=in_, func=ACTF.Identity,
                                     bias=zero_col[0:out.shape[0], :],
                                     scale=1.0)

            for j in range(ROWS):
                par = j % 2          # parity: partition half of out_pair
                pb = 64 * par
                if par == 0:
                    out_pair = outp.tile([P, N], BF16, tag="out_pair")

                # ---- input DMAs (contiguous per partition) ----
                g_sb = gp.tile([P, NT], F32, tag="g_sb")
                nc.sync.dma_start(out=g_sb[:],
                                  in_=g_d[j].rearrange("(p t) -> p t", p=P))
                g_row = gp.tile([1, N], BF16, tag="g_row")
                nc.sync.dma_start(out=g_row[:], in_=gb_d[j][None, :])

                xqk = xp.tile([P, NT, 2 * D], BF16, tag="xqk")
                nc.gpsimd.dma_start(out=xqk[:],
                                    in_=qk_d[j].rearrange("(p t) d -> p t d", p=P))
                xv1 = xp.tile([P, NT, D + 1], BF16, tag="xv1")
                nc.gpsimd.dma_start(out=xv1[:],
                                    in_=v1_d[j].rearrange("(p t) d -> p t d", p=P))

                # ---- one-hot + segment-sum matmuls ----
                ps_qk = segps.tile([P, C], F32, tag="ps_qk")
                ps_v1 = segps.tile([D + 1, C], F32, tag="ps_v1")
                for t in range(NT):
                    o_t = op_.tile([P, C + 1], BF16, tag="o_t")
                    nc.vector.tensor_scalar(out=o_t[:], in0=iota_row[:],
                                            scalar1=g_sb[:, t:t + 1],
                                            scalar2=None, op0=AOT.is_equal)
                    nc.tensor.matmul(ps_qk[:], lhsT=xqk[:, t, :],
                                     rhs=o_t[:, 0:C],
                                     start=(t == 0), stop=(t == NT - 1),
                                     skip_group_check=True)
                    nc.tensor.matmul(ps_v1[:], lhsT=xv1[:, t, :],
                                     rhs=o_t[:, 0:C],
                                     start=(t == 0), stop=(t == NT - 1),
                                     skip_group_check=True)

                # ---- small attention math ----
                # A_full[q,k] = exp(QK)[q,k]*counts[k] / sum_k exp(QK)*counts
                # and Vc*counts == Vseg (the 1/counts cancels), so the AV
                # matmul uses raw Vseg plus two extra columns: counts (the
                # denominator) and counts[0]*e0 (the acol numerator).
                counts = sp.tile([1, C], F32, tag="counts")
                nc.vector.tensor_copy(out=counts[:], in_=ps_v1[D:D + 1, :])
                ceps = sp.tile([1, C], F32, tag="ceps")
                nc.vector.tensor_scalar(out=ceps[:], in0=counts[:],
                                        scalar1=1e-20, scalar2=None,
                                        op0=AOT.add)
                w_row = sp.tile([1, C], F32, tag="w_row")
                nc.vector.reciprocal(out=w_row[:], in_=ceps[:])

                # broadcast w along partitions via K=1 f32 matmul
                ps_w = smps.tile([P, C], F32, tag="sm")
                nc.tensor.matmul(ps_w[:], lhsT=ones_row_f[:], rhs=w_row[:],
                                 start=True, stop=True, skip_group_check=True)
                w_bc = sp.tile([P, C], F32, tag="w_bc")
                acopy(w_bc[:], ps_w[:])

                # Q/K centers (transposed layout [d, c]), bf16
                qcT = sp.tile([D, C], BF16, tag="qcT")
                nc.vector.tensor_tensor(out=qcT[:], in0=ps_qk[0:D, :],
                                        in1=w_bc[0:D, :], op=AOT.mult)
                kcT = sp.tile([D, C], BF16, tag="kcT")
                nc.vector.tensor_tensor(out=kcT[:], in0=ps_qk[D:2 * D, :],
                                        in1=w_bc[D:2 * D, :], op=AOT.mult)
                # raw [Vseg | counts] in bf16 (counts < 256: exact)
                vcT = sp.tile([D + 1, C], BF16, tag="vcT")
                acopy(vcT[:], ps_v1[:])

                # S = Qc @ Kc^T  -> [129 q, 129 k] in PSUM
                ps_S = smps.tile([P, C], F32, tag="sm")
                ps_S1 = smps.tile([1, C], F32, tag="sm")
                nc.tensor.matmul(ps_S[:], lhsT=qcT[:, 0:P], rhs=kcT[:],
                                 start=True, stop=True, skip_group_check=True)
                nc.tensor.matmul(ps_S1[:], lhsT=qcT[:, P:C], rhs=kcT[:],
                                 start=True, stop=True, skip_group_check=True)

                a_exp = sp.tile([P, C], BF16, tag="a_exp")
                nc.scalar.activation(out=a_exp[:], in_=ps_S[:], func=ACTF.Exp)
                a_expt = sp.tile([1, C], BF16, tag="a_expt")
                nc.scalar.activation(out=a_expt[:], in_=ps_S1[:], func=ACTF.Exp)

                # A^T via PE transposes (bf16 pass-through)
                ps_T1 = smps.tile([P, P], BF16, tag="sm")
                nc.tensor.transpose(ps_T1[:], in_=a_exp[:, 0:P], identity=ident_bf[:])
                ps_T2 = smps.tile([1, P], BF16, tag="sm")
                nc.tensor.transpose(ps_T2[:], in_=a_exp[:, P:C], identity=ident_bf[:])
                ps_T3 = smps.tile([P, 1], BF16, tag="sm")
                nc.tensor.transpose(ps_T3[:], in_=a_expt[:, 0:P],
                                    identity=ident_bf[0:1, 0:1])
                at_m = sp.tile([P, C], BF16, tag="at_m")
                acopy(at_m[:, 0:P], ps_T1[:])
                acopy(at_m[:, P:C], ps_T3[:])
                at_t = sp.tile([1, C], BF16, tag="at_t")
                acopy(at_t[:, 0:P], ps_T2[:])
                nc.vector.tensor_copy(out=at_t[:, P:C], in_=a_expt[:, P:C])

                # [Vseg | counts] with c on partitions via PE transposes
                ps_T5 = smps.tile([P, D + 1], BF16, tag="sm")
                nc.tensor.transpose(ps_T5[:], in_=vcT[:, 0:P],
                                    identity=ident_bf[0:D + 1, 0:D + 1])
                ps_T6 = smps.tile([1, D + 1], BF16, tag="sm")
                nc.tensor.transpose(ps_T6[:], in_=vcT[:, P:C],
                                    identity=ident_bf[0:D + 1, 0:D + 1])
                # rhs = [Vseg | counts | counts[0]*e0]  -> [c, 66]
                vc_m = sp.tile([P, D + 2], BF16, tag="vc_m")
                acopy(vc_m[:, 0:D + 1], ps_T5[:])
                vc_t = sp.tile([1, D + 2], BF16, tag="vc_t")
                acopy(vc_t[:, 0:D + 1], ps_T6[:])
                # e0 column: counts[0] at partition 0, zero elsewhere
                nc.vector.memset(vc_m[:, D + 1:D + 2], 0.0)
                nc.vector.tensor_copy(out=vc_m[0:1, D + 1:D + 2],
                                      in_=vc_m[0:1, D:D + 1])
                nc.vector.memset(vc_t[:, D + 1:D + 2], 0.0)

                # V_num = A_exp @ [Vseg | counts | c0*e0]  (unnormalized)
                ps_V = smps.tile([P, D + 2], F32, tag="sm")
                ps_V1 = smps.tile([1, D + 2], F32, tag="sm")
                nc.tensor.matmul(ps_V[:], lhsT=at_m[:, 0:P], rhs=vc_m[:],
                                 start=True, stop=False, skip_group_check=True)
                nc.tensor.matmul(ps_V[:], lhsT=at_t[:, 0:P], rhs=vc_t[:],
                                 start=False, stop=True, skip_group_check=True)
                nc.tensor.matmul(ps_V1[:], lhsT=at_m[:, P:C], rhs=vc_m[:],
                                 start=True, stop=False, skip_group_check=True)
                nc.tensor.matmul(ps_V1[:], lhsT=at_t[:, P:C], rhs=vc_t[:],
                                 start=False, stop=True, skip_group_check=True)

                # rinv = 1/denominator; acol = acol_numer * rinv
                rs = sp.tile([P, 1], F32, tag="rs")
                nc.vector.tensor_copy(out=rs[:], in_=ps_V[:, D:D + 1])
                rinv = sp.tile([P, 1], F32, tag="rinv")
                nc.vector.reciprocal(out=rinv[:], in_=rs[:])
                rst = sp.tile([1, 1], F32, tag="rst")
                nc.vector.tensor_copy(out=rst[:], in_=ps_V1[:, D:D + 1])
                rinvt = sp.tile([1, 1], F32, tag="rinvt")
                nc.vector.reciprocal(out=rinvt[:], in_=rst[:])
                nc.vector.tensor_scalar(out=acol_main[:, j:j + 1],
                                        in0=ps_V[:, D + 1:D + 2],
                                        scalar1=rinv[:],
                                        scalar2=None, op0=AOT.mult)
                nc.vector.tensor_scalar(out=acol_tail[:, j:j + 1],
                                        in0=ps_V1[:, D + 1:D + 2],
                                        scalar1=rinvt[:],
                                        scalar2=None, op0=AOT.mult)

                # V_fin[128] (bias row); V' = rinv*Vsum - bias
                vfin = sp.tile([1, D], F32, tag="vfin")
                nc.vector.tensor_scalar(out=vfin[:], in0=ps_V1[:, 0:D],
                                        scalar1=rinvt[:], scalar2=None,
                                        op0=AOT.mult)
                ps_vbc = smps.tile([P, D], F32, tag="sm")
                nc.tensor.matmul(ps_vbc[:], lhsT=ones_row_f[:], rhs=vfin[:],
                                 start=True, stop=True, skip_group_check=True)
                vbc = sp.tile([P, D], F32, tag="vbc")
                acopy(vbc[:], ps_vbc[:])
                vp = sp.tile([P, D], BF16, tag="vp")
                nc.vector.scalar_tensor_tensor(out=vp[:], in0=ps_V[:, 0:D],
                                               scalar=rinv[:], in1=vbc[:],
                                               op0=AOT.mult, op1=AOT.subtract)
                # bias as a [64,1] column at this row's parity half
                ps_bias = smps.tile([P, 1], F32, tag="sm")
                nc.tensor.transpose(ps_bias[0:D, :], in_=vfin[:],
                                    identity=ones_row_f[0:1, 0:1])
                bias_c = sp.tile([P, 1], F32, tag="bias_c")
                nc.vector.tensor_copy(out=bias_c[pb:pb + D, :], in_=ps_bias[0:D, :])

                # ---- gather: out^T[d, n] = V'^T-gather + bias ----
                for e in range(NCH):
                    ps_gb = ggps.tile([P, CH], F32, tag="gg")
                    nc.tensor.matmul(ps_gb[:], lhsT=ones_row_bf[:],
                                     rhs=g_row[:, e * CH:(e + 1) * CH],
                                     start=True, stop=True,
                                     skip_group_check=True)
                    grep = otp.tile([P, CH], BF16, tag="grep")
                    acopy(grep[:], ps_gb[:])
                    ot = otp.tile([P, CH], BF16, tag="ot")
                    nc.vector.tensor_scalar(out=ot[:], in0=grep[:],
                                            scalar1=iota_col[:], scalar2=None,
                                            op0=AOT.is_equal)
                    ps_g = ggps.tile([P, CH], F32, tag="gg")
                    nc.tensor.matmul(ps_g[pb:pb + D, :], lhsT=vp[:], rhs=ot[:],
                                     start=True, stop=True,
                                     tile_position=(0, pb),
                                     skip_group_check=True)
                    nc.scalar.activation(out=out_pair[pb:pb + D,
                                                      e * CH:(e + 1) * CH],
                                         in_=ps_g[pb:pb + D, :],
                                         func=ACTF.Identity,
                                         bias=bias_c[pb:pb + D, :], scale=1.0)

                if par == 1:
                    nc.gpsimd.dma_start(out=outT_d[j // 2], in_=out_pair[:])

            nc.sync.dma_start(out=acol_d[0:P, :], in_=acol_main[:])
            nc.sync.dma_start(out=acol_d[P:C, :], in_=acol_tail[:])

    nc.compile()
    return nc


_nc_cache = None


def _get_nc():
    global _nc_cache
    if _nc_cache is None:
        _nc_cache = build()
    return _nc_cache


def _run(inputs, trace=False):
    import ml_dtypes
    q = np.asarray(inputs["queries"], dtype=np.float32)
    k = np.asarray(inputs["keys"], dtype=np.float32)
    v = np.asarray(inputs["values"], dtype=np.float32)
    cl = np.asarray(inputs["clusters"])
    g32 = cl.astype(np.float32)
    gb = g32.astype(ml_dtypes.bfloat16)  # ids <= 128, exact in bf16

    B = NCORES * ROWS
    qk = np.concatenate([q, k], axis=-1)                     # [B, N, 128]
    v1 = np.concatenate(
        [v, np.ones((B, N, 1), np.float32)], axis=-1)        # [B, N, 65]

    nc = _get_nc()
    in_maps = []
    for i in range(NCORES):
        in_maps.append({
            "qk": np.ascontiguousarray(qk[ROWS * i:ROWS * (i + 1)]),
            "v1": np.ascontiguousarray(v1[ROWS * i:ROWS * (i + 1)]),
            "g": g32,
            "gb": gb,
        })
    r = run_bass_kernel_spmd(nc, in_maps, list(range(NCORES)), trace=trace)
    res = r.results

    out = np.empty((B, N, D), np.float32)
    acol = np.empty((B, C), np.float32)
    for i in range(NCORES):
        oT = np.asarray(res[i]["outT"], np.float32)          # [2, 128, 4096]
        out[ROWS * i:ROWS * (i + 1)] = (
            oT.reshape(2, 2, D, N).transpose(0, 1, 3, 2).reshape(ROWS, N, D))
        acol[ROWS * i:ROWS * (i + 1)] = np.asarray(res[i]["acol"], np.float32).T
    return (out, acol), r


def kernel(**inputs):
    (out, acol), _ = _run(inputs, trace=False)
    return out, acol


# revision 17
# speedup vs baseline: 1.7895x; 1.0769x over previous
"""AdaClusteringAttention Trainium2 kernel (8 NeuronCores, data-parallel).

Shard batch B=32 across 8 cores (4 rows each); batch row 4i+j uses cluster
row j, so clusters are replicated to every core.

Per batch row (N=4096 tokens, D=64, C=129 clusters):
  1. One-hot segment sums via TensorE: lhsT=[Q|K] and [V|1] bf16 tiles
     (host-packed so the cast-DMA is contiguous per partition),
     rhs = one-hot O_t [128n x 129c] built on VectorE/GpSimd
     (iota + is_equal). PSUM accumulates seg^T [d, c] + exact f32 counts.
  2. Tiny [129]-sized attention math: w=1/counts, centers, QK matmuls,
     +ln(counts) folded into the QK PSUM via K=1 matmul (the weighted
     softmax reweighting for free), exp on ScalarE, row sums, A^T via PE
     transposes, AV matmuls.
  3. Gather: V'[c] = V_fin[c] - V_fin[128] makes cluster 128 implicit
     (K=128 exactly); lhsT=V' stationary, rhs=O^T [c, n] chunks; g is
     partition-broadcast by a K=1 matmul, copied PSUM->SBUF on ScalarE,
     one-hotted on VectorE (4x bf16 mode); gather output lands transposed
     [64, n] in PSUM; the V_fin[128] bias is added during the PSUM->SBUF
     copy (ScalarE Identity+bias); DMA out transposed, host un-transposes.
"""
import os
import sys

sys.path.insert(0, "/opt/trn_rl_repo")

import numpy as np

from concourse import bass, bacc, mybir
from concourse.bass_utils import run_bass_kernel_spmd
from concourse.tile import TileContext

F32 = mybir.dt.float32
BF16 = mybir.dt.bfloat16
I32 = mybir.dt.int32
AOT = mybir.AluOpType
ACTF = mybir.ActivationFunctionType

P = 128
C = 129          # clusters
N = 4096         # tokens per batch row
D = 64
NT = N // P      # 32 contraction tiles per row
ROWS = 4         # batch rows per core
NCORES = 8
CH = 512         # gather chunk width
NCH = N // CH    # 8 chunks


def build():
    nc = bacc.Bacc("TRN2", target_bir_lowering=False, debug=False,
                   num_devices=NCORES)
    qk_d = nc.declare_dram_parameter("qk", [ROWS, N, 2 * D], F32, isOutput=False)
    v1_d = nc.declare_dram_parameter("v1", [ROWS, N, D + 1], F32, isOutput=False)
    g_d = nc.declare_dram_parameter("g", [ROWS, N], F32, isOutput=False)
    gb_d = nc.declare_dram_parameter("gb", [ROWS, N], BF16, isOutput=False)
    outT_d = nc.declare_dram_parameter("outT", [ROWS // 2, P, N], F32, isOutput=True)
    acol_d = nc.declare_dram_parameter("acol", [C, ROWS], F32, isOutput=True)

    with TileContext(nc) as tc:
        with (
            tc.tile_pool(name="const", bufs=1) as cp,
            tc.tile_pool(name="xin", bufs=2) as xp,
            tc.tile_pool(name="gin", bufs=2) as gp,
            tc.tile_pool(name="opool", bufs=2) as op_,
            tc.tile_pool(name="otpool", bufs=3) as otp,
            tc.tile_pool(name="small", bufs=2) as sp,
            tc.tile_pool(name="outp", bufs=2) as outp,
            tc.tile_pool(name="segps", bufs=1, space="PSUM") as segps,
            tc.tile_pool(name="smps", bufs=3, space="PSUM") as smps,
            tc.tile_pool(name="ggps", bufs=3, space="PSUM") as ggps,
        ):
            # ---- constants ----
            # one-hot compare row padded to an even 130 cols (4x DVE mode)
            iota_row_i = cp.tile([P, C + 1], I32)
            nc.gpsimd.iota(iota_row_i[:], pattern=[[1, C + 1]], base=0,
                           channel_multiplier=0)
            iota_row = cp.tile([P, C + 1], BF16)
            nc.vector.tensor_copy(out=iota_row[:], in_=iota_row_i[:])
            iota_col_i = cp.tile([P, 1], I32)
            nc.gpsimd.iota(iota_col_i[:], pattern=[[0, 1]], base=0,
                           channel_multiplier=1)
            iota_col = cp.tile([P, 1], F32)
            nc.vector.tensor_copy(out=iota_col[:], in_=iota_col_i[:])
            ident_bf = cp.tile([P, P], BF16)
            nc.gpsimd.memset(ident_bf[:], 0.0)
            nc.gpsimd.affine_select(out=ident_bf[:], in_=ident_bf[:],
                                    compare_op=AOT.not_equal, fill=1.0,
                                    base=0, pattern=[[-1, P]],
                                    channel_multiplier=1)
            ones_row_f = cp.tile([1, P], F32)
            nc.vector.memset(ones_row_f[:], 1.0)
            ones_row_bf = cp.tile([1, P], BF16)
            nc.vector.memset(ones_row_bf[:], 1.0)
            zero_col = cp.tile([P, 1], F32)
            nc.vector.memset(zero_col[:], 0.0)

            acol_main = cp.tile([P, ROWS], F32)
            acol_tail = cp.tile([1, ROWS], F32)

            def acopy(out, in_):
                # ScalarE copy as Identity+0-bias (avoid ACT table thrash)
                nc.scalar.activation(out=out, in_=in_, func=ACTF.Identity,
                                     bias=zero_col[0:out.shape[0], :],
                                     scale=1.0)

            for j in range(ROWS):
                par = j % 2          # parity: partition half of out_pair
                pb = 64 * par
                if par == 0:
                    out_pair = outp.tile([P, N], BF16, tag="out_pair")
                    bias_c = sp.tile([P, 1], F32, tag="bias_c")

                # ---- input DMAs (contiguous per partition) ----
                g_sb = gp.tile([P, NT], F32, tag="g_sb")
                nc.sync.dma_start(out=g_sb[:],
                                  in_=g_d[j].rearrange("(p t) -> p t", p=P))
                g_row = gp.tile([1, N], BF16, tag=f"g_row{par}")
                nc.sync.dma_start(out=g_row[:], in_=gb_d[j][None, :])

                xqk = xp.tile([P, NT, 2 * D], BF16, tag="xqk")
                nc.gpsimd.dma_start(out=xqk[:],
                                    in_=qk_d[j].rearrange("(p t) d -> p t d", p=P))
                xv1 = xp.tile([P, NT, D + 1], BF16, tag="xv1")
                nc.gpsimd.dma_start(out=xv1[:],
                                    in_=v1_d[j].rearrange("(p t) d -> p t d", p=P))

                # ---- one-hot builds first (so the matmul burst is wait-free)
                o_all = op_.tile([P, NT, C + 1], BF16, tag="o_all")
                for t in range(NT):
                    nc.vector.tensor_scalar(out=o_all[:, t, :], in0=iota_row[:],
                                            scalar1=g_sb[:, t:t + 1],
                                            scalar2=None, op0=AOT.is_equal)
                # ---- segment-sum matmuls ----
                ps_qk = segps.tile([P, C], F32, tag="ps_qk")
                ps_v1 = segps.tile([D + 1, C], F32, tag="ps_v1")
                for t in range(NT):
                    nc.tensor.matmul(ps_qk[:], lhsT=xqk[:, t, :],
                                     rhs=o_all[:, t, 0:C],
                                     start=(t == 0), stop=(t == NT - 1),
                                     skip_group_check=True)
                    nc.tensor.matmul(ps_v1[:], lhsT=xv1[:, t, :],
                                     rhs=o_all[:, t, 0:C],
                                     start=(t == 0), stop=(t == NT - 1),
                                     skip_group_check=True)

                # ---- small attention math ----
                # A_full[q,k] = exp(QK)[q,k]*counts[k] / sum_k exp(QK)*counts
                # and Vc*counts == Vseg (the 1/counts cancels), so the AV
                # matmul uses raw Vseg plus two extra columns: counts (the
                # denominator) and counts[0]*e0 (the acol numerator).
                counts = sp.tile([1, C], F32, tag="counts")
                nc.vector.tensor_copy(out=counts[:], in_=ps_v1[D:D + 1, :])
                ceps = sp.tile([1, C], F32, tag="ceps")
                nc.vector.tensor_scalar(out=ceps[:], in0=counts[:],
                                        scalar1=1e-20, scalar2=None,
                                        op0=AOT.add)
                w_row = sp.tile([1, C], F32, tag="w_row")
                nc.vector.reciprocal(out=w_row[:], in_=ceps[:])

                # broadcast w along partitions via K=1 f32 matmul
                ps_w = smps.tile([P, C], F32, tag="sm")
                nc.tensor.matmul(ps_w[:], lhsT=ones_row_f[:], rhs=w_row[:],
                                 start=True, stop=True, skip_group_check=True)
                w_bc = sp.tile([P, C], F32, tag="w_bc")
                acopy(w_bc[:], ps_w[:])

                # Q/K centers (transposed layout [d, c]), bf16
                qcT = sp.tile([D, C], BF16, tag="qcT")
                nc.vector.tensor_tensor(out=qcT[:], in0=ps_qk[0:D, :],
                                        in1=w_bc[0:D, :], op=AOT.mult)
                kcT = sp.tile([D, C], BF16, tag="kcT")
                nc.vector.tensor_tensor(out=kcT[:], in0=ps_qk[D:2 * D, :],
                                        in1=w_bc[D:2 * D, :], op=AOT.mult)
                # raw [Vseg | counts] in bf16 (counts < 256: exact)
                vcT = sp.tile([D + 1, C], BF16, tag="vcT")
                acopy(vcT[:], ps_v1[:])

                # S = Qc @ Kc^T  -> [129 q, 129 k] in PSUM
                ps_S = smps.tile([P, C], F32, tag="sm")
                ps_S1 = smps.tile([1, C], F32, tag="sm")
                nc.tensor.matmul(ps_S[:], lhsT=qcT[:, 0:P], rhs=kcT[:],
                                 start=True, stop=True, skip_group_check=True)
                nc.tensor.matmul(ps_S1[:], lhsT=qcT[:, P:C], rhs=kcT[:],
                                 start=True, stop=True, skip_group_check=True)

                a_exp = sp.tile([P, C], BF16, tag="a_exp")
                nc.scalar.activation(out=a_exp[:], in_=ps_S[:], func=ACTF.Exp)
                a_expt = sp.tile([1, C], BF16, tag="a_expt")
                nc.scalar.activation(out=a_expt[:], in_=ps_S1[:], func=ACTF.Exp)

                # A^T via PE transposes (bf16 pass-through)
                ps_T1 = smps.tile([P, P], BF16, tag="sm")
                nc.tensor.transpose(ps_T1[:], in_=a_exp[:, 0:P], identity=ident_bf[:])
                ps_T2 = smps.tile([1, P], BF16, tag="sm")
                nc.tensor.transpose(ps_T2[:], in_=a_exp[:, P:C], identity=ident_bf[:])
                ps_T3 = smps.tile([P, 1], BF16, tag="sm")
                nc.tensor.transpose(ps_T3[:], in_=a_expt[:, 0:P],
                                    identity=ident_bf[0:1, 0:1])
                at_m = sp.tile([P, C], BF16, tag="at_m")
                acopy(at_m[:, 0:P], ps_T1[:])
                acopy(at_m[:, P:C], ps_T3[:])
                at_t = sp.tile([1, C], BF16, tag="at_t")
                acopy(at_t[:, 0:P], ps_T2[:])
                nc.vector.tensor_copy(out=at_t[:, P:C], in_=a_expt[:, P:C])

                # [Vseg | counts] with c on partitions via PE transposes
                ps_T5 = smps.tile([P, D + 1], BF16, tag="sm")
                nc.tensor.transpose(ps_T5[:], in_=vcT[:, 0:P],
                                    identity=ident_bf[0:D + 1, 0:D + 1])
                ps_T6 = smps.tile([1, D + 1], BF16, tag="sm")
                nc.tensor.transpose(ps_T6[:], in_=vcT[:, P:C],
                                    identity=ident_bf[0:D + 1, 0:D + 1])
                # rhs = [Vseg | counts | counts[0]*e0]  -> [c, 66]
                vc_m = sp.tile([P, D + 2], BF16, tag="vc_m")
                acopy(vc_m[:, 0:D + 1], ps_T5[:])
                vc_t = sp.tile([1, D + 2], BF16, tag="vc_t")
                acopy(vc_t[:, 0:D + 1], ps_T6[:])
                # e0 column: counts[0] at partition 0, zero elsewhere
                nc.vector.memset(vc_m[:, D + 1:D + 2], 0.0)
                nc.vector.tensor_copy(out=vc_m[0:1, D + 1:D + 2],
                                      in_=vc_m[0:1, D:D + 1])
                nc.vector.memset(vc_t[:, D + 1:D + 2], 0.0)

                # V_num = A_exp @ [Vseg | counts | c0*e0]  (unnormalized)
                ps_V = smps.tile([P, D + 2], F32, tag="sm")
                ps_V1 = smps.tile([1, D + 2], F32, tag="sm")
                nc.tensor.matmul(ps_V[:], lhsT=at_m[:, 0:P], rhs=vc_m[:],
                                 start=True, stop=False, skip_group_check=True)
                nc.tensor.matmul(ps_V[:], lhsT=at_t[:, 0:P], rhs=vc_t[:],
                                 start=False, stop=True, skip_group_check=True)
                nc.tensor.matmul(ps_V1[:], lhsT=at_m[:, P:C], rhs=vc_m[:],
                                 start=True, stop=False, skip_group_check=True)
                nc.tensor.matmul(ps_V1[:], lhsT=at_t[:, P:C], rhs=vc_t[:],
                                 start=False, stop=True, skip_group_check=True)

                # rinv = 1/denominator; acol = acol_numer * rinv
                rs = sp.tile([P, 1], F32, tag="rs")
                nc.vector.tensor_copy(out=rs[:], in_=ps_V[:, D:D + 1])
                rinv = sp.tile([P, 1], F32, tag="rinv")
                nc.vector.reciprocal(out=rinv[:], in_=rs[:])
                rst = sp.tile([1, 1], F32, tag="rst")
                nc.vector.tensor_copy(out=rst[:], in_=ps_V1[:, D:D + 1])
                rinvt = sp.tile([1, 1], F32, tag="rinvt")
                nc.vector.reciprocal(out=rinvt[:], in_=rst[:])
                nc.vector.tensor_scalar(out=acol_main[:, j:j + 1],
                                        in0=ps_V[:, D + 1:D + 2],
                                        scalar1=rinv[:],
                                        scalar2=None, op0=AOT.mult)
                nc.vector.tensor_scalar(out=acol_tail[:, j:j + 1],
                                        in0=ps_V1[:, D + 1:D + 2],
                                        scalar1=rinvt[:],
                                        scalar2=None, op0=AOT.mult)

                # V_fin[128] (bias row); V' = rinv*Vsum - bias
                vfin = sp.tile([1, D], F32, tag="vfin")
                nc.vector.tensor_scalar(out=vfin[:], in0=ps_V1[:, 0:D],
                                        scalar1=rinvt[:], scalar2=None,
                                        op0=AOT.mult)
                ps_vbc = smps.tile([P, D], F32, tag="sm")
                nc.tensor.matmul(ps_vbc[:], lhsT=ones_row_f[:], rhs=vfin[:],
                                 start=True, stop=True, skip_group_check=True)
                vbc = sp.tile([P, D], F32, tag="vbc")
                acopy(vbc[:], ps_vbc[:])
                vp = sp.tile([P, D], BF16, tag=f"vp{par}")
                nc.vector.scalar_tensor_tensor(out=vp[:], in0=ps_V[:, 0:D],
                                               scalar=rinv[:], in1=vbc[:],
                                               op0=AOT.mult, op1=AOT.subtract)
                # bias as a [64,1] column at this row's parity half
                ps_bias = smps.tile([P, 1], F32, tag="sm")
                nc.tensor.transpose(ps_bias[0:D, :], in_=vfin[:],
                                    identity=ones_row_f[0:1, 0:1])
                nc.vector.tensor_copy(out=bias_c[pb:pb + D, :], in_=ps_bias[0:D, :])

                if par == 0:
                    vp_even, g_row_even = vp, g_row
                    continue

                # ---- pair gather: out^T[d, n] = V'^T-gather + bias ----
                # both parities land in one [128, CH] psum so the epilogue
                # copy runs at full partition width
                for e in range(NCH):
                    ps_g = ggps.tile([P, CH], F32, tag="gg")
                    for prow, (vp_r, grow_r) in enumerate(
                            [(vp_even, g_row_even), (vp, g_row)]):
                        ps_gb = ggps.tile([P, CH], F32, tag="gg")
                        nc.tensor.matmul(ps_gb[:], lhsT=ones_row_bf[:],
                                         rhs=grow_r[:, e * CH:(e + 1) * CH],
                                         start=True, stop=True,
                                         skip_group_check=True)
                        ot = otp.tile([P, CH], BF16, tag="ot")
                        nc.vector.tensor_scalar(out=ot[:], in0=ps_gb[:],
                                                scalar1=iota_col[:],
                                                scalar2=None,
                                                op0=AOT.is_equal)
                        nc.tensor.matmul(ps_g[64 * prow:64 * prow + D, :],
                                         lhsT=vp_r[:], rhs=ot[:],
                                         start=True, stop=True,
                                         tile_position=(0, 64 * prow),
                                         skip_group_check=True)
                    nc.scalar.activation(out=out_pair[:, e * CH:(e + 1) * CH],
                                         in_=ps_g[:],
                                         func=ACTF.Identity,
                                         bias=bias_c[:], scale=1.0)

                nc.gpsimd.dma_start(out=outT_d[j // 2], in_=out_pair[:])

            nc.sync.dma_start(out=acol_d[0:P, :], in_=acol_main[:])
            nc.sync.dma_start(out=acol_d[P:C, :], in_=acol_tail[:])

    nc.compile()
    return nc


_nc_cache = None


def _get_nc():
    global _nc_cache
    if _nc_cache is None:
        _nc_cache = build()
    return _nc_cache


def _run(inputs, trace=False):
    import ml_dtypes
    q = np.asarray(inputs["queries"], dtype=np.float32)
    k = np.asarray(inputs["keys"], dtype=np.float32)
    v = np.asarray(inputs["values"], dtype=np.float32)
    cl = np.asarray(inputs["clusters"])
    g32 = cl.astype(np.float32)
    gb = g32.astype(ml_dtypes.bfloat16)  # ids <= 128, exact in bf16

    B = NCORES * ROWS
    qk = np.concatenate([q, k], axis=-1)                     # [B, N, 128]
    v1 = np.concatenate(
        [v, np.ones((B, N, 1), np.float32)], axis=-1)        # [B, N, 65]

    nc = _get_nc()
    in_maps = []
    for i in range(NCORES):
        in_maps.append({
            "qk": np.ascontiguousarray(qk[ROWS * i:ROWS * (i + 1)]),
            "v1": np.ascontiguousarray(v1[ROWS * i:ROWS * (i + 1)]),
            "g": g32,
            "gb": gb,
        })
    r = run_bass_kernel_spmd(nc, in_maps, list(range(NCORES)), trace=trace)
    res = r.results

    out = np.empty((B, N, D), np.float32)
    acol = np.empty((B, C), np.float32)
    for i in range(NCORES):
        oT = np.asarray(res[i]["outT"], np.float32)          # [2, 128, 4096]
        out[ROWS * i:ROWS * (i + 1)] = (
            oT.reshape(2, 2, D, N).transpose(0, 1, 3, 2).reshape(ROWS, N, D))
        acol[ROWS * i:ROWS * (i + 1)] = np.asarray(res[i]["acol"], np.float32).T
    return (out, acol), r


def kernel(**inputs):
    (out, acol), _ = _run(inputs, trace=False)
    return out, acol


# revision 18
# speedup vs baseline: 2.1830x; 1.2199x over previous
"""AdaClusteringAttention Trainium2 kernel (8 NeuronCores, data-parallel).

Shard batch B=32 across 8 cores (4 rows each); batch row 4i+j uses cluster
row j, so clusters are replicated to every core.

Per batch row (N=4096 tokens, D=64, C=129 clusters):
  1. One-hot segment sums via TensorE: lhsT=[Q|K] and [V|1] bf16 tiles
     (host-packed so the cast-DMA is contiguous per partition),
     rhs = one-hot O_t [128n x 129c] built on VectorE/GpSimd
     (iota + is_equal). PSUM accumulates seg^T [d, c] + exact f32 counts.
  2. Tiny [129]-sized attention math: w=1/counts, centers, QK matmuls,
     +ln(counts) folded into the QK PSUM via K=1 matmul (the weighted
     softmax reweighting for free), exp on ScalarE, row sums, A^T via PE
     transposes, AV matmuls.
  3. Gather: V'[c] = V_fin[c] - V_fin[128] makes cluster 128 implicit
     (K=128 exactly); lhsT=V' stationary, rhs=O^T [c, n] chunks; g is
     partition-broadcast by a K=1 matmul, copied PSUM->SBUF on ScalarE,
     one-hotted on VectorE (4x bf16 mode); gather output lands transposed
     [64, n] in PSUM; the V_fin[128] bias is added during the PSUM->SBUF
     copy (ScalarE Identity+bias); DMA out transposed, host un-transposes.
"""
import os
import sys

sys.path.insert(0, "/opt/trn_rl_repo")

import numpy as np

from concourse import bass, bacc, mybir
from concourse.bass_utils import run_bass_kernel_spmd
from concourse.tile import TileContext

F32 = mybir.dt.float32
BF16 = mybir.dt.bfloat16
I32 = mybir.dt.int32
AOT = mybir.AluOpType
ACTF = mybir.ActivationFunctionType

P = 128
C = 129          # clusters
N = 4096         # tokens per batch row
D = 64
NT = N // P      # 32 contraction tiles per row
ROWS = 4         # batch rows per core
NCORES = 8
CH = 512         # gather chunk width
NCH = N // CH    # 8 chunks


def build():
    nc = bacc.Bacc("TRN2", target_bir_lowering=False, debug=False,
                   num_devices=NCORES)
    qk_d = nc.declare_dram_parameter("qk", [ROWS, N, 2 * D], F32, isOutput=False)
    v1_d = nc.declare_dram_parameter("v1", [ROWS, N, D + 1], F32, isOutput=False)
    g_d = nc.declare_dram_parameter("g", [ROWS, N], F32, isOutput=False)
    gb_d = nc.declare_dram_parameter("gb", [ROWS, N], BF16, isOutput=False)
    outT_d = nc.declare_dram_parameter("outT", [ROWS // 2, P, N], F32, isOutput=True)
    acol_d = nc.declare_dram_parameter("acol", [C, ROWS], F32, isOutput=True)

    with TileContext(nc) as tc:
        with (
            tc.tile_pool(name="const", bufs=1) as cp,
            tc.tile_pool(name="xin", bufs=2) as xp,
            tc.tile_pool(name="gin", bufs=2) as gp,
            tc.tile_pool(name="opool", bufs=2) as op_,
            tc.tile_pool(name="otpool", bufs=3) as otp,
            tc.tile_pool(name="small", bufs=2) as sp,
            tc.tile_pool(name="outp", bufs=2) as outp,
            tc.tile_pool(name="segps", bufs=1, space="PSUM") as segps,
            tc.tile_pool(name="smps", bufs=3, space="PSUM") as smps,
            tc.tile_pool(name="ggps", bufs=3, space="PSUM") as ggps,
        ):
            # ---- constants ----
            # one-hot compare row padded to an even 130 cols (4x DVE mode)
            iota_row_i = cp.tile([P, 136], I32)
            nc.gpsimd.iota(iota_row_i[:], pattern=[[1, 136]], base=0,
                           channel_multiplier=0)
            iota_row = cp.tile([P, 136], BF16)
            nc.vector.tensor_copy(out=iota_row[:], in_=iota_row_i[:])
            iota_col_i = cp.tile([P, 1], I32)
            nc.gpsimd.iota(iota_col_i[:], pattern=[[0, 1]], base=0,
                           channel_multiplier=1)
            iota_col = cp.tile([P, 1], F32)
            nc.vector.tensor_copy(out=iota_col[:], in_=iota_col_i[:])
            ident_bf = cp.tile([P, P], BF16)
            nc.gpsimd.memset(ident_bf[:], 0.0)
            nc.gpsimd.affine_select(out=ident_bf[:], in_=ident_bf[:],
                                    compare_op=AOT.not_equal, fill=1.0,
                                    base=0, pattern=[[-1, P]],
                                    channel_multiplier=1)
            ones_row_f = cp.tile([1, P], F32)
            nc.vector.memset(ones_row_f[:], 1.0)
            ones_row_bf = cp.tile([1, P], BF16)
            nc.vector.memset(ones_row_bf[:], 1.0)
            zero_col = cp.tile([P, 1], F32)
            nc.vector.memset(zero_col[:], 0.0)

            acol_main = cp.tile([P, ROWS], F32)
            acol_tail = cp.tile([1, ROWS], F32)

            def acopy(out, in_):
                # ScalarE copy as Identity+0-bias (avoid ACT table thrash)
                nc.scalar.activation(out=out, in_=in_, func=ACTF.Identity,
                                     bias=zero_col[0:out.shape[0], :],
                                     scale=1.0)

            for j in range(ROWS):
                par = j % 2          # parity: partition half of out_pair
                pb = 64 * par
                if par == 0:
                    out_pair = outp.tile([P, N], BF16, tag="out_pair")
                    bias_c = sp.tile([P, 1], F32, tag="bias_c")

                # ---- input DMAs (contiguous per partition) ----
                g_sb = gp.tile([P, NT], F32, tag="g_sb")
                nc.sync.dma_start(out=g_sb[:],
                                  in_=g_d[j].rearrange("(p t) -> p t", p=P))
                g_row = gp.tile([1, N], BF16, tag=f"g_row{par}")
                nc.sync.dma_start(out=g_row[:], in_=gb_d[j][None, :])

                xqk = xp.tile([P, NT, 2 * D], BF16, tag="xqk")
                nc.gpsimd.dma_start(out=xqk[:],
                                    in_=qk_d[j].rearrange("(p t) d -> p t d", p=P))
                xv1 = xp.tile([P, NT, D + 1], BF16, tag="xv1")
                nc.gpsimd.dma_start(out=xv1[:],
                                    in_=v1_d[j].rearrange("(p t) d -> p t d", p=P))

                # ---- one-hot builds first (so the matmul burst is wait-free)
                o_all = op_.tile([P, NT, 136], BF16, tag="o_all")
                for t in range(NT):
                    nc.vector.tensor_scalar(out=o_all[:, t, :], in0=iota_row[:],
                                            scalar1=g_sb[:, t:t + 1],
                                            scalar2=None, op0=AOT.is_equal)
                # ---- segment-sum matmuls ----
                ps_qk = segps.tile([P, C], F32, tag="ps_qk")
                ps_v1 = segps.tile([D + 1, C], F32, tag="ps_v1")
                for t in range(NT):
                    nc.tensor.matmul(ps_qk[:], lhsT=xqk[:, t, :],
                                     rhs=o_all[:, t, 0:C],
                                     start=(t == 0), stop=(t == NT - 1),
                                     skip_group_check=True)
                    nc.tensor.matmul(ps_v1[:], lhsT=xv1[:, t, :],
                                     rhs=o_all[:, t, 0:C],
                                     start=(t == 0), stop=(t == NT - 1),
                                     skip_group_check=True)

                # ---- small attention math ----
                # A_full[q,k] = exp(QK)[q,k]*counts[k] / sum_k exp(QK)*counts
                # and Vc*counts == Vseg (the 1/counts cancels), so the AV
                # matmul uses raw Vseg plus two extra columns: counts (the
                # denominator) and counts[0]*e0 (the acol numerator).
                counts = sp.tile([1, C], F32, tag="counts")
                nc.vector.tensor_copy(out=counts[:], in_=ps_v1[D:D + 1, :])
                ceps = sp.tile([1, C], F32, tag="ceps")
                nc.vector.tensor_scalar(out=ceps[:], in0=counts[:],
                                        scalar1=1e-20, scalar2=None,
                                        op0=AOT.add)
                w_row = sp.tile([1, C], F32, tag="w_row")
                nc.vector.reciprocal(out=w_row[:], in_=ceps[:])

                # broadcast w along partitions via K=1 f32 matmul
                ps_w = smps.tile([P, C], F32, tag="sm")
                nc.tensor.matmul(ps_w[:], lhsT=ones_row_f[:], rhs=w_row[:],
                                 start=True, stop=True, skip_group_check=True)
                w_bc = sp.tile([P, C], F32, tag="w_bc")
                acopy(w_bc[:], ps_w[:])

                # Q/K centers (transposed layout [d, c]), bf16
                qcT = sp.tile([D, C], BF16, tag="qcT")
                nc.vector.tensor_tensor(out=qcT[:], in0=ps_qk[0:D, :],
                                        in1=w_bc[0:D, :], op=AOT.mult)
                kcT = sp.tile([D, C], BF16, tag="kcT")
                nc.vector.tensor_tensor(out=kcT[:], in0=ps_qk[D:2 * D, :],
                                        in1=w_bc[D:2 * D, :], op=AOT.mult)
                # raw [Vseg | counts] in bf16 (counts < 256: exact)
                vcT = sp.tile([D + 1, C], BF16, tag="vcT")
                acopy(vcT[:], ps_v1[:])

                # S = Qc @ Kc^T  -> [129 q, 129 k] in PSUM
                ps_S = smps.tile([P, C], F32, tag="sm")
                ps_S1 = smps.tile([1, C], F32, tag="sm")
                nc.tensor.matmul(ps_S[:], lhsT=qcT[:, 0:P], rhs=kcT[:],
                                 start=True, stop=True, skip_group_check=True)
                nc.tensor.matmul(ps_S1[:], lhsT=qcT[:, P:C], rhs=kcT[:],
                                 start=True, stop=True, skip_group_check=True)

                a_exp = sp.tile([P, C], BF16, tag="a_exp")
                nc.scalar.activation(out=a_exp[:], in_=ps_S[:], func=ACTF.Exp)
                a_expt = sp.tile([1, C], BF16, tag="a_expt")
                nc.scalar.activation(out=a_expt[:], in_=ps_S1[:], func=ACTF.Exp)

                # A^T via PE transposes (bf16 pass-through)
                ps_T1 = smps.tile([P, P], BF16, tag="sm")
                nc.tensor.transpose(ps_T1[:], in_=a_exp[:, 0:P], identity=ident_bf[:])
                ps_T2 = smps.tile([1, P], BF16, tag="sm")
                nc.tensor.transpose(ps_T2[:], in_=a_exp[:, P:C], identity=ident_bf[:])
                ps_T3 = smps.tile([P, 1], BF16, tag="sm")
                nc.tensor.transpose(ps_T3[:], in_=a_expt[:, 0:P],
                                    identity=ident_bf[0:1, 0:1])
                at_m = sp.tile([P, C], BF16, tag="at_m")
                acopy(at_m[:, 0:P], ps_T1[:])
                acopy(at_m[:, P:C], ps_T3[:])
                at_t = sp.tile([1, C], BF16, tag="at_t")
                acopy(at_t[:, 0:P], ps_T2[:])
                nc.vector.tensor_copy(out=at_t[:, P:C], in_=a_expt[:, P:C])

                # [Vseg | counts] with c on partitions via PE transposes
                ps_T5 = smps.tile([P, D + 1], BF16, tag="sm")
                nc.tensor.transpose(ps_T5[:], in_=vcT[:, 0:P],
                                    identity=ident_bf[0:D + 1, 0:D + 1])
                ps_T6 = smps.tile([1, D + 1], BF16, tag="sm")
                nc.tensor.transpose(ps_T6[:], in_=vcT[:, P:C],
                                    identity=ident_bf[0:D + 1, 0:D + 1])
                # rhs = [Vseg | counts | counts[0]*e0]  -> [c, 66]
                vc_m = sp.tile([P, D + 2], BF16, tag="vc_m")
                acopy(vc_m[:, 0:D + 1], ps_T5[:])
                vc_t = sp.tile([1, D + 2], BF16, tag="vc_t")
                acopy(vc_t[:, 0:D + 1], ps_T6[:])
                # e0 column: counts[0] at partition 0, zero elsewhere
                nc.vector.memset(vc_m[:, D + 1:D + 2], 0.0)
                nc.vector.tensor_copy(out=vc_m[0:1, D + 1:D + 2],
                                      in_=vc_m[0:1, D:D + 1])
                nc.vector.memset(vc_t[:, D + 1:D + 2], 0.0)

                # V_num = A_exp @ [Vseg | counts | c0*e0]  (unnormalized)
                ps_V = smps.tile([P, D + 2], F32, tag="sm")
                ps_V1 = smps.tile([1, D + 2], F32, tag="sm")
                nc.tensor.matmul(ps_V[:], lhsT=at_m[:, 0:P], rhs=vc_m[:],
                                 start=True, stop=False, skip_group_check=True)
                nc.tensor.matmul(ps_V[:], lhsT=at_t[:, 0:P], rhs=vc_t[:],
                                 start=False, stop=True, skip_group_check=True)
                nc.tensor.matmul(ps_V1[:], lhsT=at_m[:, P:C], rhs=vc_m[:],
                                 start=True, stop=False, skip_group_check=True)
                nc.tensor.matmul(ps_V1[:], lhsT=at_t[:, P:C], rhs=vc_t[:],
                                 start=False, stop=True, skip_group_check=True)

                # rinv = 1/denominator; acol = acol_numer * rinv
                rs = sp.tile([P, 1], F32, tag="rs")
                nc.vector.tensor_copy(out=rs[:], in_=ps_V[:, D:D + 1])
                rinv = sp.tile([P, 1], F32, tag="rinv")
                nc.vector.reciprocal(out=rinv[:], in_=rs[:])
                rst = sp.tile([1, 1], F32, tag="rst")
                nc.vector.tensor_copy(out=rst[:], in_=ps_V1[:, D:D + 1])
                rinvt = sp.tile([1, 1], F32, tag="rinvt")
                nc.vector.reciprocal(out=rinvt[:], in_=rst[:])
                nc.vector.tensor_scalar(out=acol_main[:, j:j + 1],
                                        in0=ps_V[:, D + 1:D + 2],
                                        scalar1=rinv[:],
                                        scalar2=None, op0=AOT.mult)
                nc.vector.tensor_scalar(out=acol_tail[:, j:j + 1],
                                        in0=ps_V1[:, D + 1:D + 2],
                                        scalar1=rinvt[:],
                                        scalar2=None, op0=AOT.mult)

                # V_fin[128] (bias row); V' = rinv*Vsum - bias
                vfin = sp.tile([1, D], F32, tag="vfin")
                nc.vector.tensor_scalar(out=vfin[:], in0=ps_V1[:, 0:D],
                                        scalar1=rinvt[:], scalar2=None,
                                        op0=AOT.mult)
                ps_vbc = smps.tile([P, D], F32, tag="sm")
                nc.tensor.matmul(ps_vbc[:], lhsT=ones_row_f[:], rhs=vfin[:],
                                 start=True, stop=True, skip_group_check=True)
                vbc = sp.tile([P, D], F32, tag="vbc")
                acopy(vbc[:], ps_vbc[:])
                vp = sp.tile([P, D], BF16, tag=f"vp{par}")
                nc.vector.scalar_tensor_tensor(out=vp[:], in0=ps_V[:, 0:D],
                                               scalar=rinv[:], in1=vbc[:],
                                               op0=AOT.mult, op1=AOT.subtract)
                # bias as a [64,1] column at this row's parity half
                ps_bias = smps.tile([P, 1], F32, tag="sm")
                nc.tensor.transpose(ps_bias[0:D, :], in_=vfin[:],
                                    identity=ones_row_f[0:1, 0:1])
                nc.vector.tensor_copy(out=bias_c[pb:pb + D, :], in_=ps_bias[0:D, :])

                if par == 0:
                    vp_even, g_row_even = vp, g_row
                    continue

                # ---- pair gather: out^T[d, n] = V'^T-gather + bias ----
                # both parities land in one [128, CH] psum so the epilogue
                # copy runs at full partition width
                for e in range(NCH):
                    ps_g = ggps.tile([P, CH], F32, tag="gg")
                    for prow, (vp_r, grow_r) in enumerate(
                            [(vp_even, g_row_even), (vp, g_row)]):
                        ps_gb = ggps.tile([P, CH], F32, tag="gg")
                        nc.tensor.matmul(ps_gb[:], lhsT=ones_row_bf[:],
                                         rhs=grow_r[:, e * CH:(e + 1) * CH],
                                         start=True, stop=True,
                                         skip_group_check=True)
                        ot = otp.tile([P, CH], BF16, tag="ot")
                        nc.vector.tensor_scalar(out=ot[:], in0=ps_gb[:],
                                                scalar1=iota_col[:],
                                                scalar2=None,
                                                op0=AOT.is_equal)
                        nc.tensor.matmul(ps_g[64 * prow:64 * prow + D, :],
                                         lhsT=vp_r[:], rhs=ot[:],
                                         start=True, stop=True,
                                         tile_position=(0, 64 * prow),
                                         skip_group_check=True)
                    nc.scalar.activation(out=out_pair[:, e * CH:(e + 1) * CH],
                                         in_=ps_g[:],
                                         func=ACTF.Identity,
                                         bias=bias_c[:], scale=1.0)

                nc.gpsimd.dma_start(out=outT_d[j // 2, :, 0:N // 2],
                                    in_=out_pair[:, 0:N // 2])
                nc.gpsimd.dma_start(out=outT_d[j // 2, :, N // 2:N],
                                    in_=out_pair[:, N // 2:N])

            nc.sync.dma_start(out=acol_d[0:P, :], in_=acol_main[:])
            nc.sync.dma_start(out=acol_d[P:C, :], in_=acol_tail[:])

    nc.compile()
    return nc


_nc_cache = None


def _get_nc():
    global _nc_cache
    if _nc_cache is None:
        _nc_cache = build()
    return _nc_cache


def _run(inputs, trace=False):
    import ml_dtypes
    q = np.asarray(inputs["queries"], dtype=np.float32)
    k = np.asarray(inputs["keys"], dtype=np.float32)
    v = np.asarray(inputs["values"], dtype=np.float32)
    cl = np.asarray(inputs["clusters"])
    g32 = cl.astype(np.float32)
    gb = g32.astype(ml_dtypes.bfloat16)  # ids <= 128, exact in bf16

    B = NCORES * ROWS
    qk = np.concatenate([q, k], axis=-1)                     # [B, N, 128]
    v1 = np.concatenate(
        [v, np.ones((B, N, 1), np.float32)], axis=-1)        # [B, N, 65]

    nc = _get_nc()
    in_maps = []
    for i in range(NCORES):
        in_maps.append({
            "qk": np.ascontiguousarray(qk[ROWS * i:ROWS * (i + 1)]),
            "v1": np.ascontiguousarray(v1[ROWS * i:ROWS * (i + 1)]),
            "g": g32,
            "gb": gb,
        })
    r = run_bass_kernel_spmd(nc, in_maps, list(range(NCORES)), trace=trace)
    res = r.results

    out = np.empty((B, N, D), np.float32)
    acol = np.empty((B, C), np.float32)
    for i in range(NCORES):
        oT = np.asarray(res[i]["outT"], np.float32)          # [2, 128, 4096]
        out[ROWS * i:ROWS * (i + 1)] = (
            oT.reshape(2, 2, D, N).transpose(0, 1, 3, 2).reshape(ROWS, N, D))
        acol[ROWS * i:ROWS * (i + 1)] = np.asarray(res[i]["acol"], np.float32).T
    return (out, acol), r


def kernel(**inputs):
    (out, acol), _ = _run(inputs, trace=False)
    return out, acol


# revision 20
# speedup vs baseline: 2.4319x; 1.1140x over previous
"""AdaClusteringAttention Trainium2 kernel (8 NeuronCores, data-parallel).

Shard batch B=32 across 8 cores (4 rows each); batch row 4i+j uses cluster
row j, so clusters are replicated to every core.

Per batch row (N=4096 tokens, D=64, C=129 clusters):
  1. One-hot segment sums via TensorE: lhsT=[Q|K] and [V|1] bf16 tiles
     (host-packed so the cast-DMA is contiguous per partition),
     rhs = one-hot O_t [128n x 129c] built on VectorE/GpSimd
     (iota + is_equal). PSUM accumulates seg^T [d, c] + exact f32 counts.
  2. Tiny [129]-sized attention math: w=1/counts, centers, QK matmuls,
     +ln(counts) folded into the QK PSUM via K=1 matmul (the weighted
     softmax reweighting for free), exp on ScalarE, row sums, A^T via PE
     transposes, AV matmuls.
  3. Gather: V'[c] = V_fin[c] - V_fin[128] makes cluster 128 implicit
     (K=128 exactly); lhsT=V' stationary, rhs=O^T [c, n] chunks; g is
     partition-broadcast by a K=1 matmul, copied PSUM->SBUF on ScalarE,
     one-hotted on VectorE (4x bf16 mode); gather output lands transposed
     [64, n] in PSUM; the V_fin[128] bias is added during the PSUM->SBUF
     copy (ScalarE Identity+bias); DMA out transposed, host un-transposes.
"""
import os
import sys

sys.path.insert(0, "/opt/trn_rl_repo")

import numpy as np

from concourse import bass, bacc, mybir
from concourse.bass_utils import run_bass_kernel_spmd
from concourse.tile import TileContext

F32 = mybir.dt.float32
BF16 = mybir.dt.bfloat16
I32 = mybir.dt.int32
AOT = mybir.AluOpType
ACTF = mybir.ActivationFunctionType

P = 128
C = 129          # clusters
N = 4096         # tokens per batch row
D = 64
NT = N // P      # 32 contraction tiles per row
ROWS = 4         # batch rows per core
NCORES = 8
CH = 512         # gather chunk width
NCH = N // CH    # 8 chunks


def build():
    nc = bacc.Bacc("TRN2", target_bir_lowering=False, debug=False,
                   num_devices=NCORES)
    qk_d = nc.declare_dram_parameter("qk", [ROWS, N, 2 * D], F32, isOutput=False)
    v1_d = nc.declare_dram_parameter("v1", [ROWS, N, D + 1], F32, isOutput=False)
    g_d = nc.declare_dram_parameter("g", [ROWS, N], F32, isOutput=False)
    gb_d = nc.declare_dram_parameter("gb", [ROWS, N], BF16, isOutput=False)
    outT_d = nc.declare_dram_parameter("outT", [ROWS // 2, P, N], F32, isOutput=True)
    acol_d = nc.declare_dram_parameter("acol", [C, ROWS], F32, isOutput=True)

    with TileContext(nc) as tc:
        with (
            tc.tile_pool(name="const", bufs=1) as cp,
            tc.tile_pool(name="xin", bufs=2) as xp,
            tc.tile_pool(name="gin", bufs=2) as gp,
            tc.tile_pool(name="opool", bufs=2) as op_,
            tc.tile_pool(name="otpool", bufs=3) as otp,
            tc.tile_pool(name="small", bufs=2) as sp,
            tc.tile_pool(name="outp", bufs=2) as outp,
            tc.tile_pool(name="segps", bufs=1, space="PSUM") as segps,
            tc.tile_pool(name="smps", bufs=3, space="PSUM") as smps,
            tc.tile_pool(name="ggps", bufs=3, space="PSUM") as ggps,
        ):
            # ---- constants ----
            # one-hot compare row padded to an even 130 cols (4x DVE mode)
            iota_row_i = cp.tile([P, 136], I32)
            nc.gpsimd.iota(iota_row_i[:], pattern=[[1, 136]], base=0,
                           channel_multiplier=0)
            iota_row = cp.tile([P, 136], BF16)
            nc.vector.tensor_copy(out=iota_row[:], in_=iota_row_i[:])
            iota_col_i = cp.tile([P, 1], I32)
            nc.gpsimd.iota(iota_col_i[:], pattern=[[0, 1]], base=0,
                           channel_multiplier=1)
            iota_col = cp.tile([P, 1], F32)
            nc.vector.tensor_copy(out=iota_col[:], in_=iota_col_i[:])
            ident_bf = cp.tile([P, P], BF16)
            nc.gpsimd.memset(ident_bf[:], 0.0)
            nc.gpsimd.affine_select(out=ident_bf[:], in_=ident_bf[:],
                                    compare_op=AOT.not_equal, fill=1.0,
                                    base=0, pattern=[[-1, P]],
                                    channel_multiplier=1)
            ones_row_f = cp.tile([1, P], F32)
            nc.vector.memset(ones_row_f[:], 1.0)
            ones_row_bf = cp.tile([1, P], BF16)
            nc.vector.memset(ones_row_bf[:], 1.0)
            zero_col = cp.tile([P, 1], F32)
            nc.vector.memset(zero_col[:], 0.0)

            acol_main = cp.tile([P, ROWS], F32)
            acol_tail = cp.tile([1, ROWS], F32)

            def acopy(out, in_):
                # ScalarE copy as Identity+0-bias (avoid ACT table thrash)
                nc.scalar.activation(out=out, in_=in_, func=ACTF.Identity,
                                     bias=zero_col[0:out.shape[0], :],
                                     scale=1.0)

            for j in range(ROWS):
                par = j % 2          # parity: partition half of out_pair
                pb = 64 * par
                if par == 0:
                    out_pair = outp.tile([P, N], BF16, tag="out_pair")
                    bias_c = sp.tile([P, 1], F32, tag="bias_c")

                # ---- input DMAs (contiguous per partition) ----
                g_sb = gp.tile([P, NT], F32, tag="g_sb")
                nc.sync.dma_start(out=g_sb[:],
                                  in_=g_d[j].rearrange("(p t) -> p t", p=P))
                g_rep = gp.tile([P, N], BF16, tag=f"g_rep{par}")
                nc.sync.dma_start(out=g_rep[:],
                                  in_=gb_d[j][None, :].to_broadcast([P, N]))

                xqk = xp.tile([P, NT, 2 * D], BF16, tag="xqk")
                nc.gpsimd.dma_start(out=xqk[:],
                                    in_=qk_d[j].rearrange("(p t) d -> p t d", p=P))
                xv1 = xp.tile([P, NT, D + 1], BF16, tag="xv1")
                nc.gpsimd.dma_start(out=xv1[:],
                                    in_=v1_d[j].rearrange("(p t) d -> p t d", p=P))

                # ---- one-hot builds first (so the matmul burst is wait-free)
                o_all = op_.tile([P, NT, 136], BF16, tag="o_all")
                for t in range(NT):
                    nc.vector.tensor_scalar(out=o_all[:, t, :], in0=iota_row[:],
                                            scalar1=g_sb[:, t:t + 1],
                                            scalar2=None, op0=AOT.is_equal)
                # ---- segment-sum matmuls ----
                ps_qk = segps.tile([P, C], F32, tag="ps_qk")
                ps_v1 = segps.tile([D + 1, C], F32, tag="ps_v1")
                for t in range(NT):
                    nc.tensor.matmul(ps_qk[:], lhsT=xqk[:, t, :],
                                     rhs=o_all[:, t, 0:C],
                                     start=(t == 0), stop=(t == NT - 1),
                                     skip_group_check=True)
                    nc.tensor.matmul(ps_v1[:], lhsT=xv1[:, t, :],
                                     rhs=o_all[:, t, 0:C],
                                     start=(t == 0), stop=(t == NT - 1),
                                     skip_group_check=True)

                # ---- small attention math ----
                # A_full[q,k] = exp(QK)[q,k]*counts[k] / sum_k exp(QK)*counts
                # and Vc*counts == Vseg (the 1/counts cancels), so the AV
                # matmul uses raw Vseg plus two extra columns: counts (the
                # denominator) and counts[0]*e0 (the acol numerator).
                counts = sp.tile([1, C], F32, tag="counts")
                nc.vector.tensor_copy(out=counts[:], in_=ps_v1[D:D + 1, :])
                ceps = sp.tile([1, C], F32, tag="ceps")
                nc.vector.tensor_scalar(out=ceps[:], in0=counts[:],
                                        scalar1=1e-20, scalar2=None,
                                        op0=AOT.add)
                w_row = sp.tile([1, C], F32, tag="w_row")
                nc.vector.reciprocal(out=w_row[:], in_=ceps[:])

                # broadcast w along partitions via K=1 f32 matmul
                ps_w = smps.tile([P, C], F32, tag="sm")
                nc.tensor.matmul(ps_w[:], lhsT=ones_row_f[:], rhs=w_row[:],
                                 start=True, stop=True, skip_group_check=True)
                w_bc = sp.tile([P, C], F32, tag="w_bc")
                acopy(w_bc[:], ps_w[:])

                # Q/K centers (transposed layout [d, c]), bf16
                qcT = sp.tile([D, C], BF16, tag="qcT")
                nc.vector.tensor_tensor(out=qcT[:], in0=ps_qk[0:D, :],
                                        in1=w_bc[0:D, :], op=AOT.mult)
                kcT = sp.tile([D, C], BF16, tag="kcT")
                nc.vector.tensor_tensor(out=kcT[:], in0=ps_qk[D:2 * D, :],
                                        in1=w_bc[D:2 * D, :], op=AOT.mult)
                # raw [Vseg | counts] in bf16 (counts < 256: exact)
                vcT = sp.tile([D + 1, C], BF16, tag="vcT")
                acopy(vcT[:], ps_v1[:])

                # S = Qc @ Kc^T  -> [129 q, 129 k] in PSUM
                ps_S = smps.tile([P, C], F32, tag="sm")
                ps_S1 = smps.tile([1, C], F32, tag="sm")
                nc.tensor.matmul(ps_S[:], lhsT=qcT[:, 0:P], rhs=kcT[:],
                                 start=True, stop=True, skip_group_check=True)
                nc.tensor.matmul(ps_S1[:], lhsT=qcT[:, P:C], rhs=kcT[:],
                                 start=True, stop=True, skip_group_check=True)

                a_exp = sp.tile([P, C], BF16, tag="a_exp")
                nc.scalar.activation(out=a_exp[:], in_=ps_S[:], func=ACTF.Exp)
                a_expt = sp.tile([1, C], BF16, tag="a_expt")
                nc.scalar.activation(out=a_expt[:], in_=ps_S1[:], func=ACTF.Exp)

                # A^T via PE transposes (bf16 pass-through)
                ps_T1 = smps.tile([P, P], BF16, tag="sm")
                nc.tensor.transpose(ps_T1[:], in_=a_exp[:, 0:P], identity=ident_bf[:])
                ps_T23 = smps.tile([P, P + 1], BF16, tag="sm")
                nc.tensor.transpose(ps_T23[0:1, 0:P], in_=a_exp[:, P:C],
                                    identity=ident_bf[:])
                nc.tensor.transpose(ps_T23[:, P:P + 1], in_=a_expt[:, 0:P],
                                    identity=ident_bf[0:1, 0:1])
                at_m = sp.tile([P, C], BF16, tag="at_m")
                acopy(at_m[:, 0:P], ps_T1[:])
                acopy(at_m[:, P:C], ps_T23[:, P:P + 1])
                at_t = sp.tile([1, C], BF16, tag="at_t")
                acopy(at_t[:, 0:P], ps_T23[0:1, 0:P])
                nc.vector.tensor_copy(out=at_t[:, P:C], in_=a_expt[:, P:C])

                # [Vseg | counts] with c on partitions via PE transposes
                ps_T56 = smps.tile([P, 2 * D + 4], BF16, tag="sm")
                nc.tensor.transpose(ps_T56[:, 0:D + 1], in_=vcT[:, 0:P],
                                    identity=ident_bf[0:D + 1, 0:D + 1])
                nc.tensor.transpose(ps_T56[0:1, D + 2:2 * D + 3],
                                    in_=vcT[:, P:C],
                                    identity=ident_bf[0:D + 1, 0:D + 1])
                # rhs = [Vseg | counts | counts[0]*e0]  -> [c, 66]
                vc_m = sp.tile([P, D + 2], BF16, tag="vc_m")
                acopy(vc_m[:, 0:D + 1], ps_T56[:, 0:D + 1])
                vc_t = sp.tile([1, D + 2], BF16, tag="vc_t")
                acopy(vc_t[:, 0:D + 1], ps_T56[0:1, D + 2:2 * D + 3])
                # e0 column: counts[0] at partition 0, zero elsewhere
                nc.vector.memset(vc_m[:, D + 1:D + 2], 0.0)
                nc.vector.tensor_copy(out=vc_m[0:1, D + 1:D + 2],
                                      in_=vc_m[0:1, D:D + 1])
                nc.vector.memset(vc_t[:, D + 1:D + 2], 0.0)

                # V_num = A_exp @ [Vseg | counts | c0*e0]  (unnormalized)
                ps_V = smps.tile([P, D + 2], F32, tag="sm")
                ps_V1 = smps.tile([1, D + 2], F32, tag="sm")
                nc.tensor.matmul(ps_V[:], lhsT=at_m[:, 0:P], rhs=vc_m[:],
                                 start=True, stop=False, skip_group_check=True)
                nc.tensor.matmul(ps_V[:], lhsT=at_t[:, 0:P], rhs=vc_t[:],
                                 start=False, stop=True, skip_group_check=True)
                nc.tensor.matmul(ps_V1[:], lhsT=at_m[:, P:C], rhs=vc_m[:],
                                 start=True, stop=False, skip_group_check=True)
                nc.tensor.matmul(ps_V1[:], lhsT=at_t[:, P:C], rhs=vc_t[:],
                                 start=False, stop=True, skip_group_check=True)

                # rinv = 1/denominator; acol = acol_numer * rinv
                rs = sp.tile([P, 1], F32, tag="rs")
                nc.vector.tensor_copy(out=rs[:], in_=ps_V[:, D:D + 1])
                rinv = sp.tile([P, 1], F32, tag="rinv")
                nc.vector.reciprocal(out=rinv[:], in_=rs[:])
                rst = sp.tile([1, 1], F32, tag="rst")
                nc.vector.tensor_copy(out=rst[:], in_=ps_V1[:, D:D + 1])
                rinvt = sp.tile([1, 1], F32, tag="rinvt")
                nc.vector.reciprocal(out=rinvt[:], in_=rst[:])
                nc.vector.tensor_scalar(out=acol_main[:, j:j + 1],
                                        in0=ps_V[:, D + 1:D + 2],
                                        scalar1=rinv[:],
                                        scalar2=None, op0=AOT.mult)
                nc.vector.tensor_scalar(out=acol_tail[:, j:j + 1],
                                        in0=ps_V1[:, D + 1:D + 2],
                                        scalar1=rinvt[:],
                                        scalar2=None, op0=AOT.mult)

                # V_fin[128] (bias row); V' = rinv*Vsum - bias
                vfin = sp.tile([1, D], F32, tag="vfin")
                nc.vector.tensor_scalar(out=vfin[:], in0=ps_V1[:, 0:D],
                                        scalar1=rinvt[:], scalar2=None,
                                        op0=AOT.mult)
                ps_vbc = smps.tile([P, D], F32, tag="sm")
                nc.tensor.matmul(ps_vbc[:], lhsT=ones_row_f[:], rhs=vfin[:],
                                 start=True, stop=True, skip_group_check=True)
                vbc = sp.tile([P, D], F32, tag="vbc")
                acopy(vbc[:], ps_vbc[:])
                vp = sp.tile([P, D], BF16, tag=f"vp{par}")
                nc.vector.scalar_tensor_tensor(out=vp[:], in0=ps_V[:, 0:D],
                                               scalar=rinv[:], in1=vbc[:],
                                               op0=AOT.mult, op1=AOT.subtract)
                # bias as a [64,1] column at this row's parity half
                ps_bias = smps.tile([P, 1], F32, tag="sm")
                nc.tensor.transpose(ps_bias[0:D, :], in_=vfin[:],
                                    identity=ones_row_f[0:1, 0:1])
                nc.vector.tensor_copy(out=bias_c[pb:pb + D, :], in_=ps_bias[0:D, :])

                if par == 0:
                    vp_even, g_rep_even = vp, g_rep
                    continue

                # ---- pair gather: out^T[d, n] = V'^T-gather + bias ----
                # both parities land in one [128, CH] psum so the epilogue
                # copy runs at full partition width
                for e in range(NCH):
                    ps_g = ggps.tile([P, CH], F32, tag="gg")
                    for prow, (vp_r, grep_r) in enumerate(
                            [(vp_even, g_rep_even), (vp, g_rep)]):
                        ot = otp.tile([P, CH], BF16, tag="ot")
                        nc.vector.tensor_scalar(
                            out=ot[:], in0=grep_r[:, e * CH:(e + 1) * CH],
                            scalar1=iota_col[:], scalar2=None,
                            op0=AOT.is_equal)
                        nc.tensor.matmul(ps_g[64 * prow:64 * prow + D, :],
                                         lhsT=vp_r[:], rhs=ot[:],
                                         start=True, stop=True,
                                         tile_position=(0, 64 * prow),
                                         skip_group_check=True)
                    nc.scalar.activation(out=out_pair[:, e * CH:(e + 1) * CH],
                                         in_=ps_g[:],
                                         func=ACTF.Identity,
                                         bias=bias_c[:], scale=1.0)

                nc.gpsimd.dma_start(out=outT_d[j // 2, :, 0:N // 2],
                                    in_=out_pair[:, 0:N // 2])
                nc.gpsimd.dma_start(out=outT_d[j // 2, :, N // 2:N],
                                    in_=out_pair[:, N // 2:N])

            nc.sync.dma_start(out=acol_d[0:P, :], in_=acol_main[:])
            nc.sync.dma_start(out=acol_d[P:C, :], in_=acol_tail[:])

    nc.compile()
    return nc


_nc_cache = None


def _get_nc():
    global _nc_cache
    if _nc_cache is None:
        _nc_cache = build()
    return _nc_cache


def _run(inputs, trace=False):
    import ml_dtypes
    q = np.asarray(inputs["queries"], dtype=np.float32)
    k = np.asarray(inputs["keys"], dtype=np.float32)
    v = np.asarray(inputs["values"], dtype=np.float32)
    cl = np.asarray(inputs["clusters"])
    g32 = cl.astype(np.float32)
    gb = g32.astype(ml_dtypes.bfloat16)  # ids <= 128, exact in bf16

    B = NCORES * ROWS
    qk = np.concatenate([q, k], axis=-1)                     # [B, N, 128]
    v1 = np.concatenate(
        [v, np.ones((B, N, 1), np.float32)], axis=-1)        # [B, N, 65]

    nc = _get_nc()
    in_maps = []
    for i in range(NCORES):
        in_maps.append({
            "qk": np.ascontiguousarray(qk[ROWS * i:ROWS * (i + 1)]),
            "v1": np.ascontiguousarray(v1[ROWS * i:ROWS * (i + 1)]),
            "g": g32,
            "gb": gb,
        })
    r = run_bass_kernel_spmd(nc, in_maps, list(range(NCORES)), trace=trace)
    res = r.results

    out = np.empty((B, N, D), np.float32)
    acol = np.empty((B, C), np.float32)
    for i in range(NCORES):
        oT = np.asarray(res[i]["outT"], np.float32)          # [2, 128, 4096]
        out[ROWS * i:ROWS * (i + 1)] = (
            oT.reshape(2, 2, D, N).transpose(0, 1, 3, 2).reshape(ROWS, N, D))
        acol[ROWS * i:ROWS * (i + 1)] = np.asarray(res[i]["acol"], np.float32).T
    return (out, acol), r


def kernel(**inputs):
    (out, acol), _ = _run(inputs, trace=False)
    return out, acol
